# revision 1
# baseline (speedup 1.0000x reference)
"""GNN sparse-attention message passing on 8 Trainium2 NeuronCores.

Strategy (edge parallelism, sharded by destination node):
- Sort edges by dst; split nodes into 8 contiguous ranges with ~equal edge counts.
- Per core, pack edges into groups of G tiles x 128 edges; each group's dst nodes
  lie in a window of <=128 consecutive node ids (dst_local = dst - group_base).
- Per tile: gather k|v rows (combined 256-col table) and q rows per edge via
  indirect DMA; score = exp(clip(sum_d k*q / 4)); msg = v * score.
- One-hot matmul (S_T[e, n] = dst_local[e]==n) accumulates [wV | Z] for the
  group's window in PSUM across the group's tiles; divide and indirect-scatter
  the 128 window rows to the per-core output slab; host concatenates slabs.
"""
import math
import numpy as np

import concourse.bass as bass
import concourse.tile as tile
from concourse import bacc, mybir
from concourse.bass_utils import run_bass_kernel_spmd

N = 50000
E = 800000
HID = 128
HEADS = 8
HD = 16
NCORES = 8
G = 12            # tiles per group
P = 128
CLIP_LO = float(np.exp(-5.0))
CLIP_HI = float(np.exp(5.0))

_cache = {}


def _pack(e_src, e_dst):
    """Sort edges by dst, shard across cores, pack into groups/tiles.

    Returns per-core arrays + layout info. All cores padded to the same
    group count Gmax and out-slab size MAXN+128.
    """
    order = np.argsort(e_dst, kind="stable")
    s = e_src[order].astype(np.int64)
    d = e_dst[order].astype(np.int64)
    deg = np.bincount(d, minlength=N)
    cum = np.cumsum(deg)
    # core boundaries in node space, ~equal edges
    bounds = [0]
    for c in range(1, NCORES):
        target = E * c // NCORES
        bounds.append(int(np.searchsorted(cum, target)))
    bounds.append(N)

    cores = []
    for c in range(NCORES):
        n0, n1 = bounds[c], bounds[c + 1]
        e0 = 0 if n0 == 0 else int(cum[n0 - 1])
        e1 = int(cum[n1 - 1]) if n1 > 0 else 0
        cs, cd = s[e0:e1], d[e0:e1]
        nodes = np.arange(n0, n1)
        ndeg = deg[n0:n1]
        groups = []   # (base, [edge index ranges]) per group
        ei = 0        # edge cursor within this core
        ni = 0        # node cursor within range
        while ni < len(nodes):
            base = int(nodes[ni])
            used = 0
            cap = G * P
            gstart = ei
            while ni < len(nodes):
                node = int(nodes[ni])
                dg = int(ndeg[ni])
                if node - base >= P:
                    break
                if used + dg > cap:
                    break
                used += dg
                ei += dg
                ni += 1
            groups.append((base, gstart, ei))
        cores.append((n0, n1, cs, cd, groups))

    Gmax = max(len(cr[4]) for cr in cores)
    MAXN = max(cr[1] - cr[0] for cr in cores)
    MAXN = ((MAXN + 127) // 128) * 128

    per_core = []
    for (n0, n1, cs, cd, groups) in cores:
        ng = len(groups)
        meta = np.zeros((Gmax, 15, P), np.int32)       # [g, col, p]
        dstl = np.full((Gmax, G, P), -1.0, np.float32)  # local dst or -1
        dstg = np.zeros((Gmax, G, P), np.int32)         # per-edge global dst (for q)
        trash = MAXN + np.arange(P, dtype=np.int32)
        for g in range(Gmax):
            if g < ng:
                base, ea, eb = groups[g]
                nxt = groups[g + 1][0] if g + 1 < ng else n1
                span = min(nxt - base, P)
                r = np.arange(P)
                meta[g, 12] = np.minimum(base + r, N - 1)           # qrow (unused now)
                meta[g, 13] = np.where(r < span, (base - n0) + r, trash)  # out rows
                es, ed = cs[ea:eb], cd[ea:eb]
                ne = eb - ea
                for t in range(G):
                    lo, hi = t * P, min((t + 1) * P, ne)
                    if lo >= ne:
                        break
                    k = hi - lo
                    meta[g, t, :k] = es[lo:hi]
                    dstl[g, t, :k] = (ed[lo:hi] - base).astype(np.float32)
                    dstg[g, t, :k] = ed[lo:hi]
            else:
                meta[g, 13] = trash
        # transpose to [P, ...] SBUF-friendly layouts
        per_core.append({
            "meta": np.ascontiguousarray(meta.transpose(2, 0, 1)).reshape(P, Gmax * 15),
            "dstl": np.ascontiguousarray(dstl.transpose(2, 0, 1)).reshape(P, Gmax * G),
            "dstg": np.ascontiguousarray(dstg.transpose(2, 0, 1)).reshape(P, Gmax * G),
            "n0": n0, "n1": n1,
        })
    return per_core, Gmax, MAXN


def _build(Gmax, MAXN):
    nc = bacc.Bacc(None, target_bir_lowering=False)
    kv = nc.declare_dram_parameter("kv", [N, 2 * HID], mybir.dt.float32, isOutput=False)
    qt = nc.declare_dram_parameter("qt", [N, HID], mybir.dt.float32, isOutput=False)
    meta = nc.declare_dram_parameter("meta", [P, Gmax * 15], mybir.dt.int32, isOutput=False)
    dstl = nc.declare_dram_parameter("dstl", [P, Gmax * G], mybir.dt.float32, isOutput=False)
    dstg = nc.declare_dram_parameter("dstg", [P, Gmax * G], mybir.dt.int32, isOutput=False)
    xout = nc.declare_dram_parameter("xout", [MAXN + P, HID], mybir.dt.float32, isOutput=True)

    f32 = mybir.dt.float32
    with tile.TileContext(nc) as tc:
        with tc.tile_pool(name="const", bufs=1) as cp, \
             tc.tile_pool(name="sbuf", bufs=3) as sb, \
             tc.tile_pool(name="meta", bufs=2) as mp, \
             tc.tile_pool(name="psum", bufs=2, space="PSUM") as ps:
            ii = cp.tile([P, P], mybir.dt.int32)
            nc.gpsimd.iota(ii[:], pattern=[[1, P]], base=0, channel_multiplier=0)
            fiota = cp.tile([P, P], f32)
            nc.vector.tensor_copy(out=fiota[:], in_=ii[:])

            for g in range(Gmax):
                meta_sb = mp.tile([P, 15], mybir.dt.int32, tag="meta")
                nc.sync.dma_start(out=meta_sb[:], in_=meta[:, g * 15:(g + 1) * 15])
                dstl_sb = mp.tile([P, G], f32, tag="dstl")
                nc.sync.dma_start(out=dstl_sb[:], in_=dstl[:, g * G:(g + 1) * G])
                dstg_sb = mp.tile([P, G], mybir.dt.int32, tag="dstg")
                nc.sync.dma_start(out=dstg_sb[:], in_=dstg[:, g * G:(g + 1) * G])

                acc = ps.tile([P, HID + HEADS], f32, space="PSUM", tag="acc")
                for t in range(G):
                    kvt = sb.tile([P, 2 * HID], f32, tag="kv")
                    nc.gpsimd.indirect_dma_start(
                        out=kvt[:], out_offset=None, in_=kv[:],
                        in_offset=bass.IndirectOffsetOnAxis(ap=meta_sb[:, t:t + 1], axis=0))
                    qe = sb.tile([P, HID], f32, tag="qe")
                    nc.gpsimd.indirect_dma_start(
                        out=qe[:], out_offset=None, in_=qt[:],
                        in_offset=bass.IndirectOffsetOnAxis(ap=dstg_sb[:, t:t + 1], axis=0))

                    st = sb.tile([P, P], f32, tag="st")
                    nc.vector.tensor_tensor(
                        out=st[:], in0=dstl_sb[:, t:t + 1].to_broadcast([P, P]),
                        in1=fiota[:], op=mybir.AluOpType.is_equal)

                    prod = sb.tile([P, HID], f32, tag="prod")
                    nc.vector.tensor_tensor(
                        out=prod[:], in0=kvt[:, :HID], in1=qe[:],
                        op=mybir.AluOpType.mult)
                    sc = sb.tile([P, HEADS], f32, tag="sc")
                    nc.vector.tensor_reduce(
                        out=sc[:], in_=prod[:].rearrange("p (h d) -> p h d", h=HEADS),
                        axis=mybir.AxisListType.X, op=mybir.AluOpType.add)
                    nc.scalar.activation(
                        out=sc[:], in_=sc[:],
                        func=mybir.ActivationFunctionType.Exp, scale=1.0 / math.sqrt(HD))
                    msgext = sb.tile([P, HID + HEADS], f32, tag="msgext")
                    nc.vector.tensor_scalar(
                        out=msgext[:, HID:], in0=sc[:],
                        scalar1=CLIP_LO, scalar2=CLIP_HI,
                        op0=mybir.AluOpType.max, op1=mybir.AluOpType.min)
                    nc.vector.tensor_tensor(
                        out=msgext[:, :HID].rearrange("p (h d) -> p h d", h=HEADS),
                        in0=kvt[:, HID:].rearrange("p (h d) -> p h d", h=HEADS),
                        in1=msgext[:, HID:][:, :, None].to_broadcast([P, HEADS, HD]),
                        op=mybir.AluOpType.mult)
                    nc.tensor.matmul(out=acc[:], lhsT=st[:], rhs=msgext[:],
                                     start=(t == 0), stop=(t == G - 1))

                zr = sb.tile([P, HEADS], f32, tag="zr")
                nc.vector.tensor_scalar(out=zr[:], in0=acc[:, HID:], scalar1=1e-6,
                                        scalar2=None, op0=mybir.AluOpType.add)
                nc.vector.reciprocal(out=zr[:], in_=zr[:])
                xsb = sb.tile([P, HID], f32, tag="xsb")
                nc.vector.tensor_tensor(
                    out=xsb[:].rearrange("p (h d) -> p h d", h=HEADS),
                    in0=acc[:, :HID].rearrange("p (h d) -> p h d", h=HEADS),
                    in1=zr[:][:, :, None].to_broadcast([P, HEADS, HD]),
                    op=mybir.AluOpType.mult)
                nc.gpsimd.indirect_dma_start(
                    out=xout[:], out_offset=bass.IndirectOffsetOnAxis(
                        ap=meta_sb[:, 13:14], axis=0),
                    in_=xsb[:], in_offset=None)
    nc.finalize()
    return nc


def kernel(q, k, v, edge_index):
    q = np.asarray(q, np.float32).reshape(N, HID)
    k = np.asarray(k, np.float32).reshape(N, HID)
    v = np.asarray(v, np.float32).reshape(N, HID)
    e = np.asarray(edge_index)
    per_core, Gmax, MAXN = _pack(e[0].astype(np.int64), e[1].astype(np.int64))

    key = (Gmax, MAXN)
    if key not in _cache:
        _cache[key] = _build(Gmax, MAXN)
    nc = _cache[key]

    kvtab = np.concatenate([k, v], axis=1)
    in_maps = []
    for pc in per_core:
        in_maps.append({"kv": kvtab, "qt": q, "meta": pc["meta"],
                        "dstl": pc["dstl"], "dstg": pc["dstg"]})
    res = run_bass_kernel_spmd(nc, in_maps, list(range(NCORES)))

    out = np.zeros((N, HID), np.float32)
    for c, pc in enumerate(per_core):
        n0, n1 = pc["n0"], pc["n1"]
        out[n0:n1] = res.results[c]["xout"][: n1 - n0]
    return out.reshape(1, N, HID)



# revision 2
# speedup vs baseline: 6.1036x; 6.1036x over previous
"""GNN sparse-attention message passing on 8 Trainium2 NeuronCores.

Strategy (edge parallelism, sharded by destination node):
- Sort edges by dst; split nodes into 8 contiguous ranges with ~equal edge counts.
- Wire traffic is the bottleneck (axon tunnel ~45MB/s), so all node tables move
  as fp16 and the k|v table is uploaded SHARDED (N/8 rows per core) and
  all-gathered on device over NeuronLink into a full DRAM table.
- q is uploaded sharded by the core's dst range (local gather, no collective).
- Per core, pack edges into groups of G tiles x 128 edges; each group's dst
  nodes lie in a window of <=128 consecutive node ids.
- Per tile: gather k|v rows (fp16 table, cast to f32 in the DMA) and q rows per
  edge via indirect DMA; score = exp(clip(sum_d k*q / 4)); msg = v * score.
- One-hot matmul (S_T[e, n] = dst_local[e]==n) accumulates [wV | Z] for the
  group's window in PSUM across the group's tiles; divide, cast fp16, and
  indirect-scatter the 128 window rows to the per-core fp16 output slab;
  host concatenates slabs.
"""
import math
import numpy as np

import concourse.bass as bass
import concourse.tile as tile
from concourse import bacc, mybir
from concourse.bass_utils import run_bass_kernel_spmd

N = 50000
E = 800000
HID = 128
HEADS = 8
HD = 16
NCORES = 8
SH = N // NCORES  # kv shard rows per core
G = 12            # tiles per group
P = 128
CLIP_LO = float(np.exp(-5.0))
CLIP_HI = float(np.exp(5.0))

_cache = {}


def _pack(e_src, e_dst):
    """Sort edges by dst, shard across cores, pack into groups/tiles.

    Returns per-core arrays + layout info. All cores padded to the same
    group count Gmax, out-slab size MAXN+128, q-shard size MAXQ.
    """
    order = np.argsort(e_dst, kind="stable")
    s = e_src[order].astype(np.int64)
    d = e_dst[order].astype(np.int64)
    deg = np.bincount(d, minlength=N)
    cum = np.cumsum(deg)
    # core boundaries in node space, ~equal edges
    bounds = [0]
    for c in range(1, NCORES):
        target = E * c // NCORES
        bounds.append(int(np.searchsorted(cum, target)))
    bounds.append(N)

    cores = []
    for c in range(NCORES):
        n0, n1 = bounds[c], bounds[c + 1]
        e0 = 0 if n0 == 0 else int(cum[n0 - 1])
        e1 = int(cum[n1 - 1]) if n1 > 0 else 0
        cs, cd = s[e0:e1], d[e0:e1]
        nodes = np.arange(n0, n1)
        ndeg = deg[n0:n1]
        groups = []   # (base, [edge index ranges]) per group
        ei = 0        # edge cursor within this core
        ni = 0        # node cursor within range
        while ni < len(nodes):
            base = int(nodes[ni])
            used = 0
            cap = G * P
            gstart = ei
            while ni < len(nodes):
                node = int(nodes[ni])
                dg = int(ndeg[ni])
                if node - base >= P:
                    break
                if used + dg > cap:
                    break
                used += dg
                ei += dg
                ni += 1
            groups.append((base, gstart, ei))
        cores.append((n0, n1, cs, cd, groups))

    Gmax = max(len(cr[4]) for cr in cores)
    MAXN = max(cr[1] - cr[0] for cr in cores)
    MAXN = ((MAXN + 127) // 128) * 128
    MAXQ = MAXN  # q shard padded to same size

    per_core = []
    for (n0, n1, cs, cd, groups) in cores:
        ng = len(groups)
        meta = np.zeros((Gmax, 15, P), np.int32)        # [g, col, p]
        dstl = np.full((Gmax, G, P), -1.0, np.float32)  # local dst in window or -1
        dstq = np.zeros((Gmax, G, P), np.int32)         # per-edge local q row (dst-n0)
        trash = MAXN + np.arange(P, dtype=np.int32)
        for g in range(Gmax):
            if g < ng:
                base, ea, eb = groups[g]
                nxt = groups[g + 1][0] if g + 1 < ng else n1
                span = min(nxt - base, P)
                r = np.arange(P)
                meta[g, 13] = np.where(r < span, (base - n0) + r, trash)  # out rows
                es, ed = cs[ea:eb], cd[ea:eb]
                ne = eb - ea
                for t in range(G):
                    lo, hi = t * P, min((t + 1) * P, ne)
                    if lo >= ne:
                        break
                    kk = hi - lo
                    meta[g, t, :kk] = es[lo:hi]
                    dstl[g, t, :kk] = (ed[lo:hi] - base).astype(np.float32)
                    dstq[g, t, :kk] = ed[lo:hi] - n0
            else:
                meta[g, 13] = trash
        # transpose to [P, ...] SBUF-friendly layouts
        per_core.append({
            "meta": np.ascontiguousarray(meta.transpose(2, 0, 1)).reshape(P, Gmax * 15),
            "dstl": np.ascontiguousarray(dstl.transpose(2, 0, 1)).reshape(P, Gmax * G),
            "dstq": np.ascontiguousarray(dstq.transpose(2, 0, 1)).reshape(P, Gmax * G),
            "n0": n0, "n1": n1,
        })
    return per_core, Gmax, MAXN, MAXQ


def _build(Gmax, MAXN, MAXQ):
    nc = bacc.Bacc(None, target_bir_lowering=False, num_devices=NCORES)
    f32 = mybir.dt.float32
    f16 = mybir.dt.float16
    kvs = nc.declare_dram_parameter("kvs", [SH, 2 * HID], f16, isOutput=False)
    qs = nc.declare_dram_parameter("qs", [MAXQ, HID], f16, isOutput=False)
    meta = nc.declare_dram_parameter("meta", [P, Gmax * 15], mybir.dt.int32, isOutput=False)
    dstl = nc.declare_dram_parameter("dstl", [P, Gmax * G], f32, isOutput=False)
    dstq = nc.declare_dram_parameter("dstq", [P, Gmax * G], mybir.dt.int32, isOutput=False)
    xout = nc.declare_dram_parameter("xout", [MAXN + P, HID], f16, isOutput=True)

    # bounce buffers for the kv all-gather (collectives can't touch I/O tensors)
    agin = nc.dram_tensor("agin", [SH, 2 * HID], f16)
    kvfull = nc.dram_tensor("kvfull", [N, 2 * HID], f16)

    with tile.TileContext(nc) as tc:
        with tc.tile_pool(name="const", bufs=1) as cp, \
             tc.tile_pool(name="sbuf", bufs=3) as sb, \
             tc.tile_pool(name="meta", bufs=2) as mp, \
             tc.tile_pool(name="psum", bufs=2, space="PSUM") as ps:
            nc.sync.dma_start(out=agin[:], in_=kvs[:])
            nc.gpsimd.collective_compute(
                "AllGather", mybir.AluOpType.bypass,
                replica_groups=[list(range(NCORES))],
                ins=[agin[:].opt()], outs=[kvfull[:].opt()])

            ii = cp.tile([P, P], mybir.dt.int32)
            nc.gpsimd.iota(ii[:], pattern=[[1, P]], base=0, channel_multiplier=0)
            fiota = cp.tile([P, P], f32)
            nc.vector.tensor_copy(out=fiota[:], in_=ii[:])

            for g in range(Gmax):
                meta_sb = mp.tile([P, 15], mybir.dt.int32, tag="meta")
                nc.sync.dma_start(out=meta_sb[:], in_=meta[:, g * 15:(g + 1) * 15])
                dstl_sb = mp.tile([P, G], f32, tag="dstl")
                nc.sync.dma_start(out=dstl_sb[:], in_=dstl[:, g * G:(g + 1) * G])
                dstq_sb = mp.tile([P, G], mybir.dt.int32, tag="dstq")
                nc.sync.dma_start(out=dstq_sb[:], in_=dstq[:, g * G:(g + 1) * G])

                acc = ps.tile([P, HID + HEADS], f32, space="PSUM", tag="acc")
                for t in range(G):
                    kvt = sb.tile([P, 2 * HID], f32, tag="kv")
                    nc.gpsimd.indirect_dma_start(
                        out=kvt[:], out_offset=None, in_=kvfull[:],
                        in_offset=bass.IndirectOffsetOnAxis(ap=meta_sb[:, t:t + 1], axis=0))
                    qe = sb.tile([P, HID], f32, tag="qe")
                    nc.gpsimd.indirect_dma_start(
                        out=qe[:], out_offset=None, in_=qs[:],
                        in_offset=bass.IndirectOffsetOnAxis(ap=dstq_sb[:, t:t + 1], axis=0))

                    st = sb.tile([P, P], f32, tag="st")
                    nc.vector.tensor_tensor(
                        out=st[:], in0=dstl_sb[:, t:t + 1].to_broadcast([P, P]),
                        in1=fiota[:], op=mybir.AluOpType.is_equal)

                    prod = sb.tile([P, HID], f32, tag="prod")
                    nc.vector.tensor_tensor(
                        out=prod[:], in0=kvt[:, :HID], in1=qe[:],
                        op=mybir.AluOpType.mult)
                    sc = sb.tile([P, HEADS], f32, tag="sc")
                    nc.vector.tensor_reduce(
                        out=sc[:], in_=prod[:].rearrange("p (h d) -> p h d", h=HEADS),
                        axis=mybir.AxisListType.X, op=mybir.AluOpType.add)
                    nc.scalar.activation(
                        out=sc[:], in_=sc[:],
                        func=mybir.ActivationFunctionType.Exp, scale=1.0 / math.sqrt(HD))
                    msgext = sb.tile([P, HID + HEADS], f32, tag="msgext")
                    nc.vector.tensor_scalar(
                        out=msgext[:, HID:], in0=sc[:],
                        scalar1=CLIP_LO, scalar2=CLIP_HI,
                        op0=mybir.AluOpType.max, op1=mybir.AluOpType.min)
                    nc.vector.tensor_tensor(
                        out=msgext[:, :HID].rearrange("p (h d) -> p h d", h=HEADS),
                        in0=kvt[:, HID:].rearrange("p (h d) -> p h d", h=HEADS),
                        in1=msgext[:, HID:][:, :, None].to_broadcast([P, HEADS, HD]),
                        op=mybir.AluOpType.mult)
                    nc.tensor.matmul(out=acc[:], lhsT=st[:], rhs=msgext[:],
                                     start=(t == 0), stop=(t == G - 1))

                zr = sb.tile([P, HEADS], f32, tag="zr")
                nc.vector.tensor_scalar(out=zr[:], in0=acc[:, HID:], scalar1=1e-6,
                                        scalar2=None, op0=mybir.AluOpType.add)
                nc.vector.reciprocal(out=zr[:], in_=zr[:])
                xsb = sb.tile([P, HID], f16, tag="xsb")
                nc.vector.tensor_tensor(
                    out=xsb[:].rearrange("p (h d) -> p h d", h=HEADS),
                    in0=acc[:, :HID].rearrange("p (h d) -> p h d", h=HEADS),
                    in1=zr[:][:, :, None].to_broadcast([P, HEADS, HD]),
                    op=mybir.AluOpType.mult)
                nc.gpsimd.indirect_dma_start(
                    out=xout[:], out_offset=bass.IndirectOffsetOnAxis(
                        ap=meta_sb[:, 13:14], axis=0),
                    in_=xsb[:], in_offset=None)
    nc.finalize()
    return nc


def kernel(q, k, v, edge_index):
    q = np.asarray(q, np.float32).reshape(N, HID)
    k = np.asarray(k, np.float32).reshape(N, HID)
    v = np.asarray(v, np.float32).reshape(N, HID)
    e = np.asarray(edge_index)
    per_core, Gmax, MAXN, MAXQ = _pack(e[0].astype(np.int64), e[1].astype(np.int64))

    key = (Gmax, MAXN, MAXQ)
    if key not in _cache:
        _cache[key] = _build(Gmax, MAXN, MAXQ)
    nc = _cache[key]

    kvtab = np.concatenate([k, v], axis=1).astype(np.float16)
    qh = q.astype(np.float16)
    in_maps = []
    for c, pc in enumerate(per_core):
        n0, n1 = pc["n0"], pc["n1"]
        qpad = np.zeros((MAXQ, HID), np.float16)
        qpad[: n1 - n0] = qh[n0:n1]
        in_maps.append({"kvs": kvtab[c * SH:(c + 1) * SH], "qs": qpad,
                        "meta": pc["meta"], "dstl": pc["dstl"], "dstq": pc["dstq"]})
    res = run_bass_kernel_spmd(nc, in_maps, list(range(NCORES)))

    out = np.zeros((N, HID), np.float32)
    for c, pc in enumerate(per_core):
        n0, n1 = pc["n0"], pc["n1"]
        out[n0:n1] = res.results[c]["xout"][: n1 - n0].astype(np.float32)
    return out.reshape(1, N, HID)


# revision 4
# speedup vs baseline: 12.1492x; 1.9905x over previous
"""GNN sparse-attention message passing on 8 Trainium2 NeuronCores.

The axon tunnel (~43MB/s) dominates end-to-end time, so the kernel is
organized around minimizing host<->device bytes:
- k table ships fp16 SHARDED (N/8 rows per core), v table int8 with one global
  scale; both are all-gathered on device over NeuronLink into full DRAM tables.
- q ships fp16 sharded by the core's destination-node range (local gather).
- Edge indices ship compressed (uint16 src, int16 local q row, int8 window
  offset, uint16 out row) and are widened on device by casting gpsimd DMAs.
- Output is quantized on device to int8 with a per-row fp16 scale; the host
  multiplies back (including the global v scale).

Compute (edge parallelism, sharded by destination node):
- Sort edges by dst; split nodes into 8 contiguous ranges with ~equal edges.
- Per core, pack edges into groups of G tiles x 128 edges; each group's dst
  nodes lie in a window of <=128 consecutive node ids.
- Per tile: gather k/v/q rows per edge via indirect DMA (dtype cast in DMA);
  score = exp(clip(sum_d k*q / 4)); msg = v * score.
- One-hot matmul (S_T[e, n] = dst_local[e]==n) accumulates [wV | Z] for the
  group's window in PSUM across the group's tiles; divide, row-quantize, and
  indirect-scatter the window rows to the per-core output slab.
"""
import math
import numpy as np

import concourse.bass as bass
import concourse.tile as tile
from concourse import bacc, mybir

N = 50000
E = 800000
HID = 128
HEADS = 8
HD = 16
NCORES = 8
SH = N // NCORES  # kv shard rows per core
G = 12            # tiles per group
P = 128
CLIP_LO = float(np.exp(-5.0))
CLIP_HI = float(np.exp(5.0))

_cache = {}


def _pack(e_src, e_dst):
    """Sort edges by dst, shard across cores, pack into groups/tiles.

    Returns per-core arrays + layout info. All cores padded to the same
    group count Gmax, out-slab size MAXN+128, q-shard size MAXQ.
    """
    order = np.argsort(e_dst, kind="stable")
    s = e_src[order].astype(np.int64)
    d = e_dst[order].astype(np.int64)
    deg = np.bincount(d, minlength=N)
    cum = np.cumsum(deg)
    # core boundaries in node space, ~equal edges
    bounds = [0]
    for c in range(1, NCORES):
        target = E * c // NCORES
        bounds.append(int(np.searchsorted(cum, target)))
    bounds.append(N)

    cores = []
    for c in range(NCORES):
        n0, n1 = bounds[c], bounds[c + 1]
        e0 = 0 if n0 == 0 else int(cum[n0 - 1])
        e1 = int(cum[n1 - 1]) if n1 > 0 else 0
        cs, cd = s[e0:e1], d[e0:e1]
        nodes = np.arange(n0, n1)
        ndeg = deg[n0:n1]
        groups = []   # (base, edge range) per group
        ei = 0        # edge cursor within this core
        ni = 0        # node cursor within range
        while ni < len(nodes):
            base = int(nodes[ni])
            used = 0
            cap = G * P
            gstart = ei
            while ni < len(nodes):
                node = int(nodes[ni])
                dg = int(ndeg[ni])
                if node - base >= P:
                    break
                if used + dg > cap:
                    break
                used += dg
                ei += dg
                ni += 1
            groups.append((base, gstart, ei))
        cores.append((n0, n1, cs, cd, groups))

    Gmax = max(len(cr[4]) for cr in cores)
    MAXN = max(cr[1] - cr[0] for cr in cores)
    MAXN = ((MAXN + 127) // 128) * 128
    MAXQ = MAXN

    per_core = []
    for (n0, n1, cs, cd, groups) in cores:
        ng = len(groups)
        srcs = np.zeros((Gmax, G, P), np.uint16)
        dstq = np.zeros((Gmax, G, P), np.int16)   # per-edge local q row (dst-n0)
        dstl = np.full((Gmax, G, P), -1, np.int8)  # local dst in window or -1
        outr = np.zeros((Gmax, 1, P), np.uint16)
        trash = (MAXN + np.arange(P)).astype(np.uint16)
        for g in range(Gmax):
            if g < ng:
                base, ea, eb = groups[g]
                nxt = groups[g + 1][0] if g + 1 < ng else n1
                span = min(nxt - base, P)
                r = np.arange(P)
                outr[g, 0] = np.where(r < span, (base - n0) + r, trash)
                es, ed = cs[ea:eb], cd[ea:eb]
                ne = eb - ea
                for t in range(G):
                    lo, hi = t * P, min((t + 1) * P, ne)
                    if lo >= ne:
                        break
                    kk = hi - lo
                    srcs[g, t, :kk] = es[lo:hi]
                    dstl[g, t, :kk] = ed[lo:hi] - base
                    dstq[g, t, :kk] = ed[lo:hi] - n0
            else:
                outr[g, 0] = trash
        per_core.append({
            "srcs": np.ascontiguousarray(srcs.transpose(2, 0, 1)).reshape(P, Gmax * G),
            "dstq": np.ascontiguousarray(dstq.transpose(2, 0, 1)).reshape(P, Gmax * G),
            "dstl": np.ascontiguousarray(dstl.transpose(2, 0, 1)).reshape(P, Gmax * G),
            "outr": np.ascontiguousarray(outr.transpose(2, 0, 1)).reshape(P, Gmax),
            "n0": n0, "n1": n1,
        })
    return per_core, Gmax, MAXN, MAXQ


def _build(Gmax, MAXN, MAXQ):
    nc = bacc.Bacc(None, target_bir_lowering=False, num_devices=NCORES)
    f32 = mybir.dt.float32
    f16 = mybir.dt.float16
    i32 = mybir.dt.int32
    kt = nc.declare_dram_parameter("kt", [SH, HID], f16, isOutput=False)
    vt = nc.declare_dram_parameter("vt", [SH, HID], mybir.dt.int8, isOutput=False)
    qs = nc.declare_dram_parameter("qs", [MAXQ, HID], f16, isOutput=False)
    srcs = nc.declare_dram_parameter("srcs", [P, Gmax * G], mybir.dt.uint16, isOutput=False)
    dstq = nc.declare_dram_parameter("dstq", [P, Gmax * G], mybir.dt.int16, isOutput=False)
    dstl = nc.declare_dram_parameter("dstl", [P, Gmax * G], mybir.dt.int8, isOutput=False)
    outr = nc.declare_dram_parameter("outr", [P, Gmax], mybir.dt.uint16, isOutput=False)
    xout = nc.declare_dram_parameter("xout", [MAXN + P, HID], mybir.dt.int8, isOutput=True)
    xsc = nc.declare_dram_parameter("xsc", [MAXN + P, 1], f16, isOutput=True)

    # bounce buffers for the all-gathers (collectives can't touch I/O tensors)
    agk = nc.dram_tensor("agk", [SH, HID], f16)
    kfull = nc.dram_tensor("kfull", [N, HID], f16)
    agv = nc.dram_tensor("agv", [SH, HID], mybir.dt.int8)
    vfull = nc.dram_tensor("vfull", [N, HID], mybir.dt.int8)

    with tile.TileContext(nc) as tc:
        with tc.tile_pool(name="const", bufs=1) as cp, \
             tc.tile_pool(name="sbuf", bufs=3) as sb, \
             tc.tile_pool(name="meta", bufs=2) as mp, \
             tc.tile_pool(name="psum", bufs=2, space="PSUM") as ps:
            nc.sync.dma_start(out=agk[:], in_=kt[:])
            nc.sync.dma_start(out=agv[:], in_=vt[:])
            nc.gpsimd.collective_compute(
                "AllGather", mybir.AluOpType.bypass,
                replica_groups=[list(range(NCORES))],
                ins=[agk[:].opt()], outs=[kfull[:].opt()])
            nc.gpsimd.collective_compute(
                "AllGather", mybir.AluOpType.bypass,
                replica_groups=[list(range(NCORES))],
                ins=[agv[:].opt()], outs=[vfull[:].opt()])

            ii = cp.tile([P, P], i32)
            nc.gpsimd.iota(ii[:], pattern=[[1, P]], base=0, channel_multiplier=0)
            fiota = cp.tile([P, P], f32)
            nc.vector.tensor_copy(out=fiota[:], in_=ii[:])

            for g in range(Gmax):
                srcs_sb = mp.tile([P, G], i32, tag="srcs")
                nc.gpsimd.dma_start(out=srcs_sb[:], in_=srcs[:, g * G:(g + 1) * G])
                dstq_sb = mp.tile([P, G], i32, tag="dstq")
                nc.gpsimd.dma_start(out=dstq_sb[:], in_=dstq[:, g * G:(g + 1) * G])
                dstl_sb = mp.tile([P, G], f32, tag="dstl")
                nc.gpsimd.dma_start(out=dstl_sb[:], in_=dstl[:, g * G:(g + 1) * G])
                outr_sb = mp.tile([P, 1], i32, tag="outr")
                nc.gpsimd.dma_start(out=outr_sb[:], in_=outr[:, g:g + 1])

                acc = ps.tile([P, HID + HEADS], f32, space="PSUM", tag="acc")
                for t in range(G):
                    ke = sb.tile([P, HID], f32, tag="ke")
                    nc.gpsimd.indirect_dma_start(
                        out=ke[:], out_offset=None, in_=kfull[:],
                        in_offset=bass.IndirectOffsetOnAxis(ap=srcs_sb[:, t:t + 1], axis=0))
                    ve = sb.tile([P, HID], f32, tag="ve")
                    nc.gpsimd.indirect_dma_start(
                        out=ve[:], out_offset=None, in_=vfull[:],
                        in_offset=bass.IndirectOffsetOnAxis(ap=srcs_sb[:, t:t + 1], axis=0))
                    qe = sb.tile([P, HID], f32, tag="qe")
                    nc.gpsimd.indirect_dma_start(
                        out=qe[:], out_offset=None, in_=qs[:],
                        in_offset=bass.IndirectOffsetOnAxis(ap=dstq_sb[:, t:t + 1], axis=0))

                    st = sb.tile([P, P], f32, tag="st")
                    nc.vector.tensor_tensor(
                        out=st[:], in0=dstl_sb[:, t:t + 1].to_broadcast([P, P]),
                        in1=fiota[:], op=mybir.AluOpType.is_equal)

                    prod = sb.tile([P, HID], f32, tag="prod")
                    nc.vector.tensor_tensor(
                        out=prod[:], in0=ke[:], in1=qe[:],
                        op=mybir.AluOpType.mult)
                    sc = sb.tile([P, HEADS], f32, tag="sc")
                    nc.vector.tensor_reduce(
                        out=sc[:], in_=prod[:].rearrange("p (h d) -> p h d", h=HEADS),
                        axis=mybir.AxisListType.X, op=mybir.AluOpType.add)
                    nc.scalar.activation(
                        out=sc[:], in_=sc[:],
                        func=mybir.ActivationFunctionType.Exp, scale=1.0 / math.sqrt(HD))
                    msgext = sb.tile([P, HID + HEADS], f32, tag="msgext")
                    nc.vector.tensor_scalar(
                        out=msgext[:, HID:], in0=sc[:],
                        scalar1=CLIP_LO, scalar2=CLIP_HI,
                        op0=mybir.AluOpType.max, op1=mybir.AluOpType.min)
                    nc.vector.tensor_tensor(
                        out=msgext[:, :HID].rearrange("p (h d) -> p h d", h=HEADS),
                        in0=ve[:].rearrange("p (h d) -> p h d", h=HEADS),
                        in1=msgext[:, HID:][:, :, None].to_broadcast([P, HEADS, HD]),
                        op=mybir.AluOpType.mult)
                    nc.tensor.matmul(out=acc[:], lhsT=st[:], rhs=msgext[:],
                                     start=(t == 0), stop=(t == G - 1))

                zr = sb.tile([P, HEADS], f32, tag="zr")
                nc.vector.tensor_scalar(out=zr[:], in0=acc[:, HID:], scalar1=1e-6,
                                        scalar2=None, op0=mybir.AluOpType.add)
                nc.vector.reciprocal(out=zr[:], in_=zr[:])
                xsb = sb.tile([P, HID], f32, tag="xsb")
                nc.vector.tensor_tensor(
                    out=xsb[:].rearrange("p (h d) -> p h d", h=HEADS),
                    in0=acc[:, :HID].rearrange("p (h d) -> p h d", h=HEADS),
                    in1=zr[:][:, :, None].to_broadcast([P, HEADS, HD]),
                    op=mybir.AluOpType.mult)

                # per-row int8 quantization: scale = absmax/127, guarded vs 0
                xab = sb.tile([P, HID], f32, tag="xab")
                nc.scalar.activation(out=xab[:], in_=xsb[:],
                                     func=mybir.ActivationFunctionType.Abs)
                rmax = sb.tile([P, 1], f32, tag="rmax")
                nc.vector.tensor_reduce(
                    out=rmax[:], in_=xab[:],
                    axis=mybir.AxisListType.X, op=mybir.AluOpType.max)
                nc.vector.tensor_scalar(out=rmax[:], in0=rmax[:], scalar1=1e-30,
                                        scalar2=None, op0=mybir.AluOpType.add)
                rinv = sb.tile([P, 1], f32, tag="rinv")
                nc.vector.reciprocal(out=rinv[:], in_=rmax[:])
                nc.vector.tensor_scalar(out=rinv[:], in0=rinv[:], scalar1=127.0,
                                        scalar2=None, op0=mybir.AluOpType.mult)
                xq8 = sb.tile([P, HID], mybir.dt.int8, tag="xq8")
                nc.vector.tensor_tensor(
                    out=xq8[:], in0=xsb[:], in1=rinv[:].to_broadcast([P, HID]),
                    op=mybir.AluOpType.mult)
                xscf = sb.tile([P, 1], f16, tag="xscf")
                nc.vector.tensor_scalar(out=xscf[:], in0=rmax[:], scalar1=1.0 / 127.0,
                                        scalar2=None, op0=mybir.AluOpType.mult)

                nc.gpsimd.indirect_dma_start(
                    out=xout[:], out_offset=bass.IndirectOffsetOnAxis(
                        ap=outr_sb[:, 0:1], axis=0),
                    in_=xq8[:], in_offset=None)
                nc.gpsimd.indirect_dma_start(
                    out=xsc[:], out_offset=bass.IndirectOffsetOnAxis(
                        ap=outr_sb[:, 0:1], axis=0),
                    in_=xscf[:], in_offset=None)
    nc.finalize()
    return nc


def _make_runner(nc):
    """Cached PJRT runner: jitted shard_map over 8 cores with device-created
    donated zero output buffers (avoids uploading zeros over the tunnel)."""
    import jax
    import jax.numpy as jnp
    from jax.experimental.shard_map import shard_map
    from jax.sharding import Mesh, PartitionSpec, NamedSharding
    from concourse.bass2jax import (
        _bass_exec_p, install_neuronx_cc_hook, partition_id_tensor)

    install_neuronx_cc_hook()
    partition_name = nc.partition_id_tensor.name if nc.partition_id_tensor else None

    in_names, out_names, out_avals = [], [], []
    for alloc in nc.m.functions[0].allocations:
        if not isinstance(alloc, mybir.MemoryLocationSet):
            continue
        name = alloc.memorylocations[0].name
        if alloc.kind == "ExternalInput":
            if name != partition_name:
                in_names.append(name)
        elif alloc.kind == "ExternalOutput":
            shape = tuple(alloc.tensor_shape)
            dtype = mybir.dt.np(alloc.dtype)
            out_names.append(name)
            out_avals.append(jax.core.ShapedArray(shape, dtype))

    dbg_name = None
    if nc.dbg_addr is not None:
        assert not nc.dbg_callbacks
        dbg_name = nc.dbg_addr.name
        if dbg_name in in_names:
            dbg_name = None  # already counted

    n_params = len(in_names)
    n_outs = len(out_names)
    all_names = list(in_names) + list(out_names)
    if partition_name is not None:
        all_names.append(partition_name)
    donate = tuple(range(n_params, n_params + n_outs))

    def _body(*args):
        operands = list(args)
        if partition_name is not None:
            operands.append(partition_id_tensor())
        outs = _bass_exec_p.bind(
            *operands,
            out_avals=tuple(out_avals),
            in_names=tuple(all_names),
            out_names=tuple(out_names),
            lowering_input_output_aliases=(),
            sim_require_finite=True,
            sim_require_nnan=True,
            nc=nc,
        )
        return tuple(outs)

    devices = jax.devices()[:NCORES]
    mesh = Mesh(np.asarray(devices), ("core",))
    in_specs = (PartitionSpec("core"),) * (n_params + n_outs)
    out_specs = (PartitionSpec("core"),) * n_outs
    sharded = jax.jit(
        shard_map(_body, mesh=mesh, in_specs=in_specs, out_specs=out_specs,
                  check_rep=False),
        donate_argnums=donate, keep_unused=True)

    zspec = NamedSharding(mesh, PartitionSpec("core"))
    zshapes = [(NCORES * a.shape[0], *a.shape[1:]) for a in out_avals]
    zdtypes = [a.dtype for a in out_avals]
    zeros_fn = jax.jit(
        lambda: tuple(jnp.zeros(s, d) for s, d in zip(zshapes, zdtypes)),
        out_shardings=tuple(zspec for _ in out_avals))

    def run(concat_in_map):
        ins = [concat_in_map[name] for name in in_names]
        outs = sharded(*ins, *zeros_fn())
        return {name: outs[i] for i, name in enumerate(out_names)}

    return run, in_names


def kernel(q, k, v, edge_index):
    q = np.asarray(q, np.float32).reshape(N, HID)
    k = np.asarray(k, np.float32).reshape(N, HID)
    v = np.asarray(v, np.float32).reshape(N, HID)
    e = np.asarray(edge_index)
    per_core, Gmax, MAXN, MAXQ = _pack(e[0].astype(np.int64), e[1].astype(np.int64))

    key = (Gmax, MAXN, MAXQ)
    if key not in _cache:
        nc = _build(Gmax, MAXN, MAXQ)
        _cache[key] = _make_runner(nc)
    run, _ = _cache[key]

    gscale = float(np.abs(v).max()) / 127.0
    vq = np.rint(v * (1.0 / gscale)).astype(np.int8)   # |v|/gscale <= 127 by construction
    kh = k.astype(np.float16)

    qs_all = np.zeros((NCORES * MAXQ, HID), np.float16)
    for c, pc in enumerate(per_core):
        n0, n1 = pc["n0"], pc["n1"]
        qs_all[c * MAXQ: c * MAXQ + (n1 - n0)] = q[n0:n1].astype(np.float16)

    concat = {
        "kt": kh,                 # node order == concat of 8 shards
        "vt": vq,
        "qs": qs_all,
        "srcs": np.concatenate([pc["srcs"] for pc in per_core], axis=0),
        "dstq": np.concatenate([pc["dstq"] for pc in per_core], axis=0),
        "dstl": np.concatenate([pc["dstl"] for pc in per_core], axis=0),
        "outr": np.concatenate([pc["outr"] for pc in per_core], axis=0),
    }
    outs = run(concat)
    xq = np.asarray(outs["xout"]).reshape(NCORES, MAXN + P, HID)
    xs = np.asarray(outs["xsc"]).reshape(NCORES, MAXN + P, 1)

    out = np.zeros((N, HID), np.float32)
    for c, pc in enumerate(per_core):
        n0, n1 = pc["n0"], pc["n1"]
        nn = n1 - n0
        out[n0:n1] = (xq[c, :nn].astype(np.float32)
                      * (xs[c, :nn].astype(np.float32) * gscale))
    return out.reshape(1, N, HID)


# revision 5
# speedup vs baseline: 14.2639x; 1.1741x over previous
"""GNN sparse-attention message passing on 8 Trainium2 NeuronCores.

The axon tunnel (~43MB/s) dominates end-to-end time, so the kernel is
organized around minimizing host<->device bytes and overlapping host work
with the upload:
- k ships fp16 SHARDED (N/8 rows per core); v and q ship int8 (v: one global
  scale; q: per-row scale + fp16 scale vector). k and v are all-gathered on
  device over NeuronLink into full DRAM tables; q is sharded by the core's
  fixed destination-node range (local gather, no collective).
- Node tables don't depend on edge packing (fixed core bounds), so their
  device_put is dispatched asynchronously BEFORE the host packs edges.
- Edge indices ship compressed (uint16 src, int8 window offset, uint16 out row
  and group base) and are widened on device by casting gpsimd DMAs; the local
  q row per edge is reconstructed on device as window_offset + group_base.
- Output is quantized on device to int8 with a per-row fp16 scale; the host
  multiplies back (including the global v scale).

Compute (edge parallelism, sharded by destination node):
- Sort edges by dst; core c owns dst range [c*6250, (c+1)*6250).
- Per core, pack edges into groups of G tiles x 128 edges; each group's dst
  nodes lie in a window of <=128 consecutive node ids.
- Per tile: gather k/v/q rows + q scale per edge via indirect DMA (dtype cast
  in DMA); score = qscale * exp(clip(sum_d k*q / 4)) ... (scale applied before
  exp); msg = v * score.
- One-hot matmul (S_T[e, n] = dst_local[e]==n) accumulates [wV | Z] for the
  group's window in PSUM across the group's tiles; divide, row-quantize, and
  indirect-scatter the window rows to the per-core output slab.
"""
import math
import numpy as np

import concourse.bass as bass
import concourse.tile as tile
from concourse import bacc, mybir

N = 50000
E = 800000
HID = 128
HEADS = 8
HD = 16
NCORES = 8
SH = N // NCORES          # nodes per core (fixed bounds)
MAXQ = ((SH + 127) // 128) * 128
MAXN = MAXQ
G = 12                    # tiles per group
P = 128
CLIP_LO = float(np.exp(-5.0))
CLIP_HI = float(np.exp(5.0))

_cache = {}


def _pack(e_src, e_dst):
    """Sort edges by dst, shard across fixed core ranges, pack into groups.

    Vectorized: loops only over cores x groups (~500 iterations).
    """
    d32 = e_dst.astype(np.int32)
    order = np.argsort(d32, kind="stable")
    s = e_src.astype(np.int32)[order]
    d = d32[order]
    deg = np.bincount(d, minlength=N)
    cum = np.concatenate([[0], np.cumsum(deg)])  # cum[n] = edges with dst < n

    # greedy group boundaries per core: window <=P nodes, <=G*P edges
    core_groups = []
    for c in range(NCORES):
        n0, n1 = c * SH, (c + 1) * SH
        bases = []
        ni = n0
        while ni < n1:
            bases.append(ni)
            cap_node = min(ni + P, n1)
            cap_edge = int(np.searchsorted(cum, cum[ni] + G * P, side="right")) - 1
            ni = max(ni + 1, min(cap_node, cap_edge))
        core_groups.append(bases)
    Gmax = max(len(b) for b in core_groups)

    per_core = []
    r = np.arange(P)
    for c in range(NCORES):
        n0, n1 = c * SH, (c + 1) * SH
        bases = np.asarray(core_groups[c], np.int64)
        ng = len(bases)
        nxt = np.concatenate([bases[1:], [n1]])
        e0s, e1s = cum[bases], cum[nxt]          # edge ranges per group
        ne = e1s - e0s

        srcs = np.zeros((Gmax, G * P), np.uint16)
        dstl = np.full((Gmax, G * P), -1, np.int8)
        outr = np.zeros((Gmax, P), np.uint16)
        gbase = np.zeros((Gmax, 1, P), np.uint16)
        trash = (MAXN + r).astype(np.uint16)

        # flat slot index for every edge of this core in one shot
        ce0, ce1 = cum[n0], cum[n1]
        es = s[ce0:ce1]
        ed = d[ce0:ce1]
        slot = np.repeat(np.arange(ng) * (G * P) - (e0s - ce0), ne) \
            + np.arange(ce1 - ce0)
        flat_s = srcs.reshape(-1)
        flat_l = dstl.reshape(-1)
        flat_s[slot] = es.astype(np.uint16)
        flat_l[slot] = (ed - np.repeat(bases, ne)).astype(np.int8)

        span = np.minimum(nxt - bases, P)                       # [ng]
        rows = (bases[:, None] - n0) + r[None, :]               # [ng, P]
        outr[:ng] = np.where(r[None, :] < span[:, None], rows, trash[None, :])
        outr[ng:] = trash[None, :]
        gbase[:ng, 0, :] = (bases[:, None] - n0).astype(np.uint16)

        per_core.append({
            "srcs": np.ascontiguousarray(
                srcs.reshape(Gmax, G, P).transpose(2, 0, 1)).reshape(P, Gmax * G),
            "dstl": np.ascontiguousarray(
                dstl.reshape(Gmax, G, P).transpose(2, 0, 1)).reshape(P, Gmax * G),
            "outr": np.ascontiguousarray(
                outr.reshape(Gmax, 1, P).transpose(2, 0, 1)).reshape(P, Gmax),
            "gbase": np.ascontiguousarray(
                gbase.transpose(2, 0, 1)).reshape(P, Gmax),
            "n0": n0, "n1": n1,
        })
    return per_core, Gmax


def _build(Gmax):
    nc = bacc.Bacc(None, target_bir_lowering=False, num_devices=NCORES)
    f32 = mybir.dt.float32
    f16 = mybir.dt.float16
    i32 = mybir.dt.int32
    i8 = mybir.dt.int8
    u16 = mybir.dt.uint16
    kt = nc.declare_dram_parameter("kt", [SH, HID], f16, isOutput=False)
    vt = nc.declare_dram_parameter("vt", [SH, HID], i8, isOutput=False)
    qs = nc.declare_dram_parameter("qs", [MAXQ, HID], i8, isOutput=False)
    qsc = nc.declare_dram_parameter("qsc", [MAXQ, 1], f16, isOutput=False)
    srcs = nc.declare_dram_parameter("srcs", [P, Gmax * G], u16, isOutput=False)
    dstl = nc.declare_dram_parameter("dstl", [P, Gmax * G], i8, isOutput=False)
    outr = nc.declare_dram_parameter("outr", [P, Gmax], u16, isOutput=False)
    gbase = nc.declare_dram_parameter("gbase", [P, Gmax], u16, isOutput=False)
    xout = nc.declare_dram_parameter("xout", [MAXN + P, HID], i8, isOutput=True)
    xsc = nc.declare_dram_parameter("xsc", [MAXN + P, 1], f16, isOutput=True)

    # bounce buffers for the all-gathers (collectives can't touch I/O tensors)
    agk = nc.dram_tensor("agk", [SH, HID], f16)
    kfull = nc.dram_tensor("kfull", [N, HID], f16)
    agv = nc.dram_tensor("agv", [SH, HID], i8)
    vfull = nc.dram_tensor("vfull", [N, HID], i8)

    with tile.TileContext(nc) as tc:
        with tc.tile_pool(name="const", bufs=1) as cp, \
             tc.tile_pool(name="sbuf", bufs=3) as sb, \
             tc.tile_pool(name="meta", bufs=2) as mp, \
             tc.tile_pool(name="psum", bufs=2, space="PSUM") as ps:
            nc.sync.dma_start(out=agk[:], in_=kt[:])
            nc.sync.dma_start(out=agv[:], in_=vt[:])
            nc.gpsimd.collective_compute(
                "AllGather", mybir.AluOpType.bypass,
                replica_groups=[list(range(NCORES))],
                ins=[agk[:].opt()], outs=[kfull[:].opt()])
            nc.gpsimd.collective_compute(
                "AllGather", mybir.AluOpType.bypass,
                replica_groups=[list(range(NCORES))],
                ins=[agv[:].opt()], outs=[vfull[:].opt()])

            ii = cp.tile([P, P], i32)
            nc.gpsimd.iota(ii[:], pattern=[[1, P]], base=0, channel_multiplier=0)
            fiota = cp.tile([P, P], f32)
            nc.vector.tensor_copy(out=fiota[:], in_=ii[:])

            for g in range(Gmax):
                srcs_sb = mp.tile([P, G], i32, tag="srcs")
                nc.gpsimd.dma_start(out=srcs_sb[:], in_=srcs[:, g * G:(g + 1) * G])
                dstlf_sb = mp.tile([P, G], f32, tag="dstlf")
                nc.gpsimd.dma_start(out=dstlf_sb[:], in_=dstl[:, g * G:(g + 1) * G])
                dstli_sb = mp.tile([P, G], i32, tag="dstli")
                nc.gpsimd.dma_start(out=dstli_sb[:], in_=dstl[:, g * G:(g + 1) * G])
                outr_sb = mp.tile([P, 1], i32, tag="outr")
                nc.gpsimd.dma_start(out=outr_sb[:], in_=outr[:, g:g + 1])
                gb_sb = mp.tile([P, 1], i32, tag="gb")
                nc.gpsimd.dma_start(out=gb_sb[:], in_=gbase[:, g:g + 1])

                # local q row per edge = window offset + group base, clamped >=0
                dstq_sb = mp.tile([P, G], i32, tag="dstq")
                nc.vector.tensor_tensor(
                    out=dstq_sb[:], in0=dstli_sb[:],
                    in1=gb_sb[:].to_broadcast([P, G]), op=mybir.AluOpType.add)
                nc.vector.tensor_scalar(out=dstq_sb[:], in0=dstq_sb[:],
                                        scalar1=0, scalar2=None,
                                        op0=mybir.AluOpType.max)

                acc = ps.tile([P, HID + HEADS], f32, space="PSUM", tag="acc")
                for t in range(G):
                    ke = sb.tile([P, HID], f32, tag="ke")
                    nc.gpsimd.indirect_dma_start(
                        out=ke[:], out_offset=None, in_=kfull[:],
                        in_offset=bass.IndirectOffsetOnAxis(ap=srcs_sb[:, t:t + 1], axis=0))
                    ve = sb.tile([P, HID], f32, tag="ve")
                    nc.gpsimd.indirect_dma_start(
                        out=ve[:], out_offset=None, in_=vfull[:],
                        in_offset=bass.IndirectOffsetOnAxis(ap=srcs_sb[:, t:t + 1], axis=0))
                    qe = sb.tile([P, HID], f32, tag="qe")
                    nc.gpsimd.indirect_dma_start(
                        out=qe[:], out_offset=None, in_=qs[:],
                        in_offset=bass.IndirectOffsetOnAxis(ap=dstq_sb[:, t:t + 1], axis=0))
                    qsce = sb.tile([P, 1], f32, tag="qsce")
                    nc.gpsimd.indirect_dma_start(
                        out=qsce[:], out_offset=None, in_=qsc[:],
                        in_offset=bass.IndirectOffsetOnAxis(ap=dstq_sb[:, t:t + 1], axis=0))

                    st = sb.tile([P, P], f32, tag="st")
                    nc.vector.tensor_tensor(
                        out=st[:], in0=dstlf_sb[:, t:t + 1].to_broadcast([P, P]),
                        in1=fiota[:], op=mybir.AluOpType.is_equal)

                    prod = sb.tile([P, HID], f32, tag="prod")
                    nc.vector.tensor_tensor(
                        out=prod[:], in0=ke[:], in1=qe[:],
                        op=mybir.AluOpType.mult)
                    sc = sb.tile([P, HEADS], f32, tag="sc")
                    nc.vector.tensor_reduce(
                        out=sc[:], in_=prod[:].rearrange("p (h d) -> p h d", h=HEADS),
                        axis=mybir.AxisListType.X, op=mybir.AluOpType.add)
                    # apply per-dst q scale before exp
                    nc.vector.tensor_tensor(
                        out=sc[:], in0=sc[:], in1=qsce[:].to_broadcast([P, HEADS]),
                        op=mybir.AluOpType.mult)
                    nc.scalar.activation(
                        out=sc[:], in_=sc[:],
                        func=mybir.ActivationFunctionType.Exp, scale=1.0 / math.sqrt(HD))
                    msgext = sb.tile([P, HID + HEADS], f32, tag="msgext")
                    nc.vector.tensor_scalar(
                        out=msgext[:, HID:], in0=sc[:],
                        scalar1=CLIP_LO, scalar2=CLIP_HI,
                        op0=mybir.AluOpType.max, op1=mybir.AluOpType.min)
                    nc.vector.tensor_tensor(
                        out=msgext[:, :HID].rearrange("p (h d) -> p h d", h=HEADS),
                        in0=ve[:].rearrange("p (h d) -> p h d", h=HEADS),
                        in1=msgext[:, HID:][:, :, None].to_broadcast([P, HEADS, HD]),
                        op=mybir.AluOpType.mult)
                    nc.tensor.matmul(out=acc[:], lhsT=st[:], rhs=msgext[:],
                                     start=(t == 0), stop=(t == G - 1))

                zr = sb.tile([P, HEADS], f32, tag="zr")
                nc.vector.tensor_scalar(out=zr[:], in0=acc[:, HID:], scalar1=1e-6,
                                        scalar2=None, op0=mybir.AluOpType.add)
                nc.vector.reciprocal(out=zr[:], in_=zr[:])
                xsb = sb.tile([P, HID], f32, tag="xsb")
                nc.vector.tensor_tensor(
                    out=xsb[:].rearrange("p (h d) -> p h d", h=HEADS),
                    in0=acc[:, :HID].rearrange("p (h d) -> p h d", h=HEADS),
                    in1=zr[:][:, :, None].to_broadcast([P, HEADS, HD]),
                    op=mybir.AluOpType.mult)

                # per-row int8 quantization: scale = absmax/127, guarded vs 0
                xab = sb.tile([P, HID], f32, tag="xab")
                nc.scalar.activation(out=xab[:], in_=xsb[:],
                                     func=mybir.ActivationFunctionType.Abs)
                rmax = sb.tile([P, 1], f32, tag="rmax")
                nc.vector.tensor_reduce(
                    out=rmax[:], in_=xab[:],
                    axis=mybir.AxisListType.X, op=mybir.AluOpType.max)
                nc.vector.tensor_scalar(out=rmax[:], in0=rmax[:], scalar1=1e-30,
                                        scalar2=None, op0=mybir.AluOpType.add)
                rinv = sb.tile([P, 1], f32, tag="rinv")
                nc.vector.reciprocal(out=rinv[:], in_=rmax[:])
                nc.vector.tensor_scalar(out=rinv[:], in0=rinv[:], scalar1=127.0,
                                        scalar2=None, op0=mybir.AluOpType.mult)
                xq8 = sb.tile([P, HID], i8, tag="xq8")
                nc.vector.tensor_tensor(
                    out=xq8[:], in0=xsb[:], in1=rinv[:].to_broadcast([P, HID]),
                    op=mybir.AluOpType.mult)
                xscf = sb.tile([P, 1], f16, tag="xscf")
                nc.vector.tensor_scalar(out=xscf[:], in0=rmax[:], scalar1=1.0 / 127.0,
                                        scalar2=None, op0=mybir.AluOpType.mult)

                nc.gpsimd.indirect_dma_start(
                    out=xout[:], out_offset=bass.IndirectOffsetOnAxis(
                        ap=outr_sb[:, 0:1], axis=0),
                    in_=xq8[:], in_offset=None)
                nc.gpsimd.indirect_dma_start(
                    out=xsc[:], out_offset=bass.IndirectOffsetOnAxis(
                        ap=outr_sb[:, 0:1], axis=0),
                    in_=xscf[:], in_offset=None)
    nc.finalize()
    return nc


def _make_runner(nc):
    """Cached PJRT runner: jitted shard_map over 8 cores with device-created
    donated zero output buffers (avoids uploading zeros over the tunnel)."""
    import jax
    import jax.numpy as jnp
    from jax.experimental.shard_map import shard_map
    from jax.sharding import Mesh, PartitionSpec, NamedSharding
    from concourse.bass2jax import (
        _bass_exec_p, install_neuronx_cc_hook, partition_id_tensor)

    install_neuronx_cc_hook()
    partition_name = nc.partition_id_tensor.name if nc.partition_id_tensor else None

    in_names, out_names, out_avals = [], [], []
    for alloc in nc.m.functions[0].allocations:
        if not isinstance(alloc, mybir.MemoryLocationSet):
            continue
        name = alloc.memorylocations[0].name
        if alloc.kind == "ExternalInput":
            if name != partition_name:
                in_names.append(name)
        elif alloc.kind == "ExternalOutput":
            shape = tuple(alloc.tensor_shape)
            dtype = mybir.dt.np(alloc.dtype)
            out_names.append(name)
            out_avals.append(jax.core.ShapedArray(shape, dtype))

    n_params = len(in_names)
    n_outs = len(out_names)
    all_names = list(in_names) + list(out_names)
    if partition_name is not None:
        all_names.append(partition_name)
    donate = tuple(range(n_params, n_params + n_outs))

    def _body(*args):
        operands = list(args)
        if partition_name is not None:
            operands.append(partition_id_tensor())
        outs = _bass_exec_p.bind(
            *operands,
            out_avals=tuple(out_avals),
            in_names=tuple(all_names),
            out_names=tuple(out_names),
            lowering_input_output_aliases=(),
            sim_require_finite=True,
            sim_require_nnan=True,
            nc=nc,
        )
        return tuple(outs)

    devices = jax.devices()[:NCORES]
    mesh = Mesh(np.asarray(devices), ("core",))
    in_specs = (PartitionSpec("core"),) * (n_params + n_outs)
    out_specs = (PartitionSpec("core"),) * n_outs
    sharded = jax.jit(
        shard_map(_body, mesh=mesh, in_specs=in_specs, out_specs=out_specs,
                  check_rep=False),
        donate_argnums=donate, keep_unused=True)

    zspec = NamedSharding(mesh, PartitionSpec("core"))
    zshapes = [(NCORES * a.shape[0], *a.shape[1:]) for a in out_avals]
    zdtypes = [a.dtype for a in out_avals]
    zeros_fn = jax.jit(
        lambda: tuple(jnp.zeros(s, d) for s, d in zip(zshapes, zdtypes)),
        out_shardings=tuple(zspec for _ in out_avals))

    def run(concat_in_map):
        ins = [concat_in_map[name] for name in in_names]
        outs = sharded(*ins, *zeros_fn())
        return {name: outs[i] for i, name in enumerate(out_names)}

    return run, zspec


def kernel(q, k, v, edge_index):
    import jax
    q = np.asarray(q, np.float32).reshape(N, HID)
    k = np.asarray(k, np.float32).reshape(N, HID)
    v = np.asarray(v, np.float32).reshape(N, HID)
    e = np.asarray(edge_index)

    if "runner" not in _cache:
        # Gmax only depends on edge packing; build eagerly with this input's
        # packing so repeated calls hit the cache.
        per_core, Gmax = _pack(e[0], e[1])
        nc = _build(Gmax)
        _cache["runner"] = (_make_runner(nc), Gmax)
        _cache["pack"] = per_core
    (run, zspec), Gmax_built = _cache["runner"]

    # ---- phase A: node tables (independent of edge packing) -> async upload
    gscale = float(np.abs(v).max()) / 127.0
    vq = np.rint(v * (1.0 / gscale)).astype(np.int8)
    kh = k.astype(np.float16)
    qabs = np.abs(q).max(axis=1, keepdims=True)
    qscale = (qabs / 127.0 + 1e-30).astype(np.float32)
    qq = np.rint(q / qscale).astype(np.int8)
    qs_all = np.zeros((NCORES * MAXQ, HID), np.int8)
    qsc_all = np.zeros((NCORES * MAXQ, 1), np.float16)
    qs_all.reshape(NCORES, MAXQ, HID)[:, :SH] = qq.reshape(NCORES, SH, HID)
    qsc_all.reshape(NCORES, MAXQ, 1)[:, :SH] = \
        qscale.astype(np.float16).reshape(NCORES, SH, 1)
    kt_dev = jax.device_put(kh, zspec)
    vt_dev = jax.device_put(vq, zspec)
    qs_dev = jax.device_put(qs_all, zspec)
    qsc_dev = jax.device_put(qsc_all, zspec)

    # ---- phase B: edge packing (overlaps with phase-A upload)
    per_core, Gmax = _pack(e[0], e[1])
    if Gmax != Gmax_built:   # unexpected input distribution: rebuild
        nc = _build(Gmax)
        _cache["runner"] = (_make_runner(nc), Gmax)
        (run, zspec), Gmax_built = _cache["runner"]

    concat = {
        "kt": kt_dev, "vt": vt_dev, "qs": qs_dev, "qsc": qsc_dev,
        "srcs": np.concatenate([pc["srcs"] for pc in per_core], axis=0),
        "dstl": np.concatenate([pc["dstl"] for pc in per_core], axis=0),
        "outr": np.concatenate([pc["outr"] for pc in per_core], axis=0),
        "gbase": np.concatenate([pc["gbase"] for pc in per_core], axis=0),
    }
    outs = run(concat)
    xq = np.asarray(outs["xout"]).reshape(NCORES, MAXN + P, HID)
    xs = np.asarray(outs["xsc"]).reshape(NCORES, MAXN + P, 1)

    out = (xq[:, :SH].astype(np.float32)
           * (xs[:, :SH].astype(np.float32) * gscale))
    return out.reshape(1, N, HID)


# revision 7
# speedup vs baseline: 16.7026x; 1.1710x over previous
"""GNN sparse-attention message passing on 8 Trainium2 NeuronCores.

The axon tunnel (~43MB/s) dominates end-to-end time, so the kernel is
organized around minimizing host<->device bytes and overlapping host work
with the upload:
- k and v ship as ONE int8 table [N, 256] SHARDED (N/8 rows per core):
  k rows per-row-scaled (scale vector ships fp16), v globally scaled. The
  table is all-gathered on device over NeuronLink into a full DRAM table.
- q ships int8 per-row-scaled, sharded by the core's fixed destination-node
  range (local gather, no collective).
- Node tables don't depend on edge packing (fixed core bounds), so their
  device_put is dispatched asynchronously BEFORE the host packs edges.
- Edge indices ship compressed (uint16 src, int8 window offset, uint16 out row
  and group base) and are widened on device by casting gpsimd DMAs; the local
  q row per edge is reconstructed on device as window_offset + group_base.
- Output is quantized on device to int8 with a per-row fp16 scale; the host
  multiplies back (including the global v scale).

Compute (edge parallelism, sharded by destination node):
- Sort edges by dst (uint16 radix argsort); core c owns dst range
  [c*6250, (c+1)*6250).
- Per core, pack edges into groups of G tiles x 128 edges; each group's dst
  nodes lie in a window of <=128 consecutive node ids.
- Per tile: gather k|v / q rows + k,q scales per edge via indirect DMA (dtype
  cast in DMA); score = exp(clip(kscale*qscale * sum_d k*q / 4)); msg = v*score.
- One-hot matmul (S_T[e, n] = dst_local[e]==n) accumulates [wV | Z] for the
  group's window in PSUM across the group's tiles; divide, row-quantize, and
  indirect-scatter the window rows to the per-core output slab.
"""
import math
import numpy as np

import concourse.bass as bass
import concourse.tile as tile
from concourse import bacc, mybir

N = 50000
E = 800000
HID = 128
HEADS = 8
HD = 16
NCORES = 8
SH = N // NCORES          # nodes per core (fixed bounds)
MAXQ = ((SH + 127) // 128) * 128
MAXN = MAXQ
G = 12                    # tiles per group
P = 128
CLIP_LO = float(np.exp(-5.0))
CLIP_HI = float(np.exp(5.0))

_cache = {}


def _pack(e_src, e_dst):
    """Sort edges by dst, shard across fixed core ranges, pack into groups.

    Vectorized: loops only over cores x groups (~500 iterations). dst fits
    uint16, where numpy's stable argsort is a 2-pass radix (~8ms for 800k).
    """
    order = np.argsort(e_dst.astype(np.uint16), kind="stable")
    s = e_src.astype(np.int32)[order]
    d = e_dst.astype(np.int32)[order]
    deg = np.bincount(d, minlength=N)
    cum = np.concatenate([[0], np.cumsum(deg)])  # cum[n] = edges with dst < n

    # greedy group boundaries per core: window <=P nodes, <=G*P edges
    core_groups = []
    for c in range(NCORES):
        n0, n1 = c * SH, (c + 1) * SH
        bases = []
        ni = n0
        while ni < n1:
            bases.append(ni)
            cap_node = min(ni + P, n1)
            cap_edge = int(np.searchsorted(cum, cum[ni] + G * P, side="right")) - 1
            ni = max(ni + 1, min(cap_node, cap_edge))
        core_groups.append(bases)
    Gmax = max(len(b) for b in core_groups)

    per_core = []
    r = np.arange(P)
    for c in range(NCORES):
        n0, n1 = c * SH, (c + 1) * SH
        bases = np.asarray(core_groups[c], np.int64)
        ng = len(bases)
        nxt = np.concatenate([bases[1:], [n1]])
        e0s, e1s = cum[bases], cum[nxt]          # edge ranges per group
        ne = e1s - e0s

        srcs = np.zeros((Gmax, G * P), np.uint16)
        dstl = np.full((Gmax, G * P), -1, np.int8)
        outr = np.zeros((Gmax, P), np.uint16)
        gbase = np.zeros((Gmax, 1, P), np.uint16)
        trash = (MAXN + r).astype(np.uint16)

        # flat slot index for every edge of this core in one shot
        ce0, ce1 = cum[n0], cum[n1]
        es = s[ce0:ce1]
        ed = d[ce0:ce1]
        slot = np.repeat(np.arange(ng) * (G * P) - (e0s - ce0), ne) \
            + np.arange(ce1 - ce0)
        flat_s = srcs.reshape(-1)
        flat_l = dstl.reshape(-1)
        flat_s[slot] = es.astype(np.uint16)
        flat_l[slot] = (ed - np.repeat(bases, ne)).astype(np.int8)

        span = np.minimum(nxt - bases, P)                       # [ng]
        rows = (bases[:, None] - n0) + r[None, :]               # [ng, P]
        outr[:ng] = np.where(r[None, :] < span[:, None], rows, trash[None, :])
        outr[ng:] = trash[None, :]
        gbase[:ng, 0, :] = (bases[:, None] - n0).astype(np.uint16)

        per_core.append({
            "srcs": np.ascontiguousarray(
                srcs.reshape(Gmax, G, P).transpose(2, 0, 1)).reshape(P, Gmax * G),
            "dstl": np.ascontiguousarray(
                dstl.reshape(Gmax, G, P).transpose(2, 0, 1)).reshape(P, Gmax * G),
            "outr": np.ascontiguousarray(
                outr.reshape(Gmax, 1, P).transpose(2, 0, 1)).reshape(P, Gmax),
            "gbase": np.ascontiguousarray(
                gbase.transpose(2, 0, 1)).reshape(P, Gmax),
            "n0": n0, "n1": n1,
        })
    return per_core, Gmax


def _build(Gmax):
    nc = bacc.Bacc(None, target_bir_lowering=False, num_devices=NCORES)
    f32 = mybir.dt.float32
    f16 = mybir.dt.float16
    i32 = mybir.dt.int32
    i8 = mybir.dt.int8
    u16 = mybir.dt.uint16
    kv8 = nc.declare_dram_parameter("kv8", [SH, 2 * HID], i8, isOutput=False)
    ksc = nc.declare_dram_parameter("ksc", [SH, 1], f16, isOutput=False)
    qs = nc.declare_dram_parameter("qs", [MAXQ, HID], i8, isOutput=False)
    qsc = nc.declare_dram_parameter("qsc", [MAXQ, 1], f16, isOutput=False)
    srcs = nc.declare_dram_parameter("srcs", [P, Gmax * G], u16, isOutput=False)
    dstl = nc.declare_dram_parameter("dstl", [P, Gmax * G], i8, isOutput=False)
    outr = nc.declare_dram_parameter("outr", [P, Gmax], u16, isOutput=False)
    gbase = nc.declare_dram_parameter("gbase", [P, Gmax], u16, isOutput=False)
    xout = nc.declare_dram_parameter("xout", [MAXN + P, HID], i8, isOutput=True)
    xsc = nc.declare_dram_parameter("xsc", [MAXN + P, 1], f16, isOutput=True)

    # bounce buffers for the all-gathers (collectives can't touch I/O tensors)
    agkv = nc.dram_tensor("agkv", [SH, 2 * HID], i8)
    kvfull = nc.dram_tensor("kvfull", [N, 2 * HID], i8)
    agks = nc.dram_tensor("agks", [SH, 1], f16)
    kscfull = nc.dram_tensor("kscfull", [N, 1], f16)

    with tile.TileContext(nc) as tc:
        with tc.tile_pool(name="const", bufs=1) as cp, \
             tc.tile_pool(name="sbuf", bufs=3) as sb, \
             tc.tile_pool(name="meta", bufs=2) as mp, \
             tc.tile_pool(name="psum", bufs=2, space="PSUM") as ps:
            nc.sync.dma_start(out=agkv[:], in_=kv8[:])
            nc.sync.dma_start(out=agks[:], in_=ksc[:])
            nc.gpsimd.collective_compute(
                "AllGather", mybir.AluOpType.bypass,
                replica_groups=[list(range(NCORES))],
                ins=[agkv[:].opt()], outs=[kvfull[:].opt()])
            nc.gpsimd.collective_compute(
                "AllGather", mybir.AluOpType.bypass,
                replica_groups=[list(range(NCORES))],
                ins=[agks[:].opt()], outs=[kscfull[:].opt()])

            ii = cp.tile([P, P], i32)
            nc.gpsimd.iota(ii[:], pattern=[[1, P]], base=0, channel_multiplier=0)
            fiota = cp.tile([P, P], f32)
            nc.vector.tensor_copy(out=fiota[:], in_=ii[:])

            for g in range(Gmax):
                srcs_sb = mp.tile([P, G], i32, tag="srcs")
                nc.gpsimd.dma_start(out=srcs_sb[:], in_=srcs[:, g * G:(g + 1) * G])
                dstlf_sb = mp.tile([P, G], f32, tag="dstlf")
                nc.gpsimd.dma_start(out=dstlf_sb[:], in_=dstl[:, g * G:(g + 1) * G])
                dstli_sb = mp.tile([P, G], i32, tag="dstli")
                nc.gpsimd.dma_start(out=dstli_sb[:], in_=dstl[:, g * G:(g + 1) * G])
                outr_sb = mp.tile([P, 1], i32, tag="outr")
                nc.gpsimd.dma_start(out=outr_sb[:], in_=outr[:, g:g + 1])
                gb_sb = mp.tile([P, 1], i32, tag="gb")
                nc.gpsimd.dma_start(out=gb_sb[:], in_=gbase[:, g:g + 1])

                # local q row per edge = window offset + group base, clamped >=0
                dstq_sb = mp.tile([P, G], i32, tag="dstq")
                nc.vector.tensor_tensor(
                    out=dstq_sb[:], in0=dstli_sb[:],
                    in1=gb_sb[:].to_broadcast([P, G]), op=mybir.AluOpType.add)
                nc.vector.tensor_scalar(out=dstq_sb[:], in0=dstq_sb[:],
                                        scalar1=0, scalar2=None,
                                        op0=mybir.AluOpType.max)

                acc = ps.tile([P, HID + HEADS], f32, space="PSUM", tag="acc")
                for t in range(G):
                    kvt = sb.tile([P, 2 * HID], f32, tag="kvt")
                    nc.gpsimd.indirect_dma_start(
                        out=kvt[:], out_offset=None, in_=kvfull[:],
                        in_offset=bass.IndirectOffsetOnAxis(ap=srcs_sb[:, t:t + 1], axis=0))
                    ksce = sb.tile([P, 1], f32, tag="ksce")
                    nc.gpsimd.indirect_dma_start(
                        out=ksce[:], out_offset=None, in_=kscfull[:],
                        in_offset=bass.IndirectOffsetOnAxis(ap=srcs_sb[:, t:t + 1], axis=0))
                    qe = sb.tile([P, HID], f32, tag="qe")
                    nc.gpsimd.indirect_dma_start(
                        out=qe[:], out_offset=None, in_=qs[:],
                        in_offset=bass.IndirectOffsetOnAxis(ap=dstq_sb[:, t:t + 1], axis=0))
                    qsce = sb.tile([P, 1], f32, tag="qsce")
                    nc.gpsimd.indirect_dma_start(
                        out=qsce[:], out_offset=None, in_=qsc[:],
                        in_offset=bass.IndirectOffsetOnAxis(ap=dstq_sb[:, t:t + 1], axis=0))

                    st = sb.tile([P, P], f32, tag="st")
                    nc.vector.tensor_tensor(
                        out=st[:], in0=dstlf_sb[:, t:t + 1].to_broadcast([P, P]),
                        in1=fiota[:], op=mybir.AluOpType.is_equal)

                    prod = sb.tile([P, HID], f32, tag="prod")
                    nc.vector.tensor_tensor(
                        out=prod[:], in0=kvt[:, :HID], in1=qe[:],
                        op=mybir.AluOpType.mult)
                    sc = sb.tile([P, HEADS], f32, tag="sc")
                    nc.vector.tensor_reduce(
                        out=sc[:], in_=prod[:].rearrange("p (h d) -> p h d", h=HEADS),
                        axis=mybir.AxisListType.X, op=mybir.AluOpType.add)
                    # apply per-src k scale * per-dst q scale before exp
                    ssc = sb.tile([P, 1], f32, tag="ssc")
                    nc.vector.tensor_tensor(
                        out=ssc[:], in0=ksce[:], in1=qsce[:],
                        op=mybir.AluOpType.mult)
                    nc.vector.tensor_tensor(
                        out=sc[:], in0=sc[:], in1=ssc[:].to_broadcast([P, HEADS]),
                        op=mybir.AluOpType.mult)
                    nc.scalar.activation(
                        out=sc[:], in_=sc[:],
                        func=mybir.ActivationFunctionType.Exp, scale=1.0 / math.sqrt(HD))
                    msgext = sb.tile([P, HID + HEADS], f32, tag="msgext")
                    nc.vector.tensor_scalar(
                        out=msgext[:, HID:], in0=sc[:],
                        scalar1=CLIP_LO, scalar2=CLIP_HI,
                        op0=mybir.AluOpType.max, op1=mybir.AluOpType.min)
                    nc.vector.tensor_tensor(
                        out=msgext[:, :HID].rearrange("p (h d) -> p h d", h=HEADS),
                        in0=kvt[:, HID:].rearrange("p (h d) -> p h d", h=HEADS),
                        in1=msgext[:, HID:][:, :, None].to_broadcast([P, HEADS, HD]),
                        op=mybir.AluOpType.mult)
                    nc.tensor.matmul(out=acc[:], lhsT=st[:], rhs=msgext[:],
                                     start=(t == 0), stop=(t == G - 1))

                zr = sb.tile([P, HEADS], f32, tag="zr")
                nc.vector.tensor_scalar(out=zr[:], in0=acc[:, HID:], scalar1=1e-6,
                                        scalar2=None, op0=mybir.AluOpType.add)
                nc.vector.reciprocal(out=zr[:], in_=zr[:])
                xsb = sb.tile([P, HID], f32, tag="xsb")
                nc.vector.tensor_tensor(
                    out=xsb[:].rearrange("p (h d) -> p h d", h=HEADS),
                    in0=acc[:, :HID].rearrange("p (h d) -> p h d", h=HEADS),
                    in1=zr[:][:, :, None].to_broadcast([P, HEADS, HD]),
                    op=mybir.AluOpType.mult)

                # per-row int8 quantization: scale = absmax/127, guarded vs 0
                xab = sb.tile([P, HID], f32, tag="xab")
                nc.scalar.activation(out=xab[:], in_=xsb[:],
                                     func=mybir.ActivationFunctionType.Abs)
                rmax = sb.tile([P, 1], f32, tag="rmax")
                nc.vector.tensor_reduce(
                    out=rmax[:], in_=xab[:],
                    axis=mybir.AxisListType.X, op=mybir.AluOpType.max)
                nc.vector.tensor_scalar(out=rmax[:], in0=rmax[:], scalar1=1e-30,
                                        scalar2=None, op0=mybir.AluOpType.add)
                rinv = sb.tile([P, 1], f32, tag="rinv")
                nc.vector.reciprocal(out=rinv[:], in_=rmax[:])
                nc.vector.tensor_scalar(out=rinv[:], in0=rinv[:], scalar1=127.0,
                                        scalar2=None, op0=mybir.AluOpType.mult)
                xq8 = sb.tile([P, HID], i8, tag="xq8")
                nc.vector.tensor_tensor(
                    out=xq8[:], in0=xsb[:], in1=rinv[:].to_broadcast([P, HID]),
                    op=mybir.AluOpType.mult)
                xscf = sb.tile([P, 1], f16, tag="xscf")
                nc.vector.tensor_scalar(out=xscf[:], in0=rmax[:], scalar1=1.0 / 127.0,
                                        scalar2=None, op0=mybir.AluOpType.mult)

                nc.gpsimd.indirect_dma_start(
                    out=xout[:], out_offset=bass.IndirectOffsetOnAxis(
                        ap=outr_sb[:, 0:1], axis=0),
                    in_=xq8[:], in_offset=None)
                nc.gpsimd.indirect_dma_start(
                    out=xsc[:], out_offset=bass.IndirectOffsetOnAxis(
                        ap=outr_sb[:, 0:1], axis=0),
                    in_=xscf[:], in_offset=None)
    nc.finalize()
    return nc


def _make_runner(nc):
    """Cached PJRT runner: jitted shard_map over 8 cores with device-created
    donated zero output buffers (avoids uploading zeros over the tunnel)."""
    import jax
    import jax.numpy as jnp
    from jax.experimental.shard_map import shard_map
    from jax.sharding import Mesh, PartitionSpec, NamedSharding
    from concourse.bass2jax import (
        _bass_exec_p, install_neuronx_cc_hook, partition_id_tensor)

    install_neuronx_cc_hook()
    partition_name = nc.partition_id_tensor.name if nc.partition_id_tensor else None

    in_names, out_names, out_avals = [], [], []
    for alloc in nc.m.functions[0].allocations:
        if not isinstance(alloc, mybir.MemoryLocationSet):
            continue
        name = alloc.memorylocations[0].name
        if alloc.kind == "ExternalInput":
            if name != partition_name:
                in_names.append(name)
        elif alloc.kind == "ExternalOutput":
            shape = tuple(alloc.tensor_shape)
            dtype = mybir.dt.np(alloc.dtype)
            out_names.append(name)
            out_avals.append(jax.core.ShapedArray(shape, dtype))

    n_params = len(in_names)
    n_outs = len(out_names)
    all_names = list(in_names) + list(out_names)
    if partition_name is not None:
        all_names.append(partition_name)
    donate = tuple(range(n_params, n_params + n_outs))

    def _body(*args):
        operands = list(args)
        if partition_name is not None:
            operands.append(partition_id_tensor())
        outs = _bass_exec_p.bind(
            *operands,
            out_avals=tuple(out_avals),
            in_names=tuple(all_names),
            out_names=tuple(out_names),
            lowering_input_output_aliases=(),
            sim_require_finite=True,
            sim_require_nnan=True,
            nc=nc,
        )
        return tuple(outs)

    devices = jax.devices()[:NCORES]
    mesh = Mesh(np.asarray(devices), ("core",))
    in_specs = (PartitionSpec("core"),) * (n_params + n_outs)
    out_specs = (PartitionSpec("core"),) * n_outs
    sharded = jax.jit(
        shard_map(_body, mesh=mesh, in_specs=in_specs, out_specs=out_specs,
                  check_rep=False),
        donate_argnums=donate, keep_unused=True)

    zspec = NamedSharding(mesh, PartitionSpec("core"))
    zshapes = [(NCORES * a.shape[0], *a.shape[1:]) for a in out_avals]
    zdtypes = [a.dtype for a in out_avals]
    zeros_fn = jax.jit(
        lambda: tuple(jnp.zeros(s, d) for s, d in zip(zshapes, zdtypes)),
        out_shardings=tuple(zspec for _ in out_avals))

    def run(concat_in_map):
        ins = [concat_in_map[name] for name in in_names]
        outs = sharded(*ins, *zeros_fn())
        return {name: outs[i] for i, name in enumerate(out_names)}

    return run, zspec


def kernel(q, k, v, edge_index):
    import jax
    q = np.asarray(q, np.float32).reshape(N, HID)
    k = np.asarray(k, np.float32).reshape(N, HID)
    v = np.asarray(v, np.float32).reshape(N, HID)
    e = np.asarray(edge_index)

    if "runner" not in _cache:
        per_core0, Gmax0 = _pack(e[0], e[1])
        nc = _build(Gmax0)
        _cache["runner"] = (_make_runner(nc), Gmax0)
    (run, zspec), Gmax_built = _cache["runner"]

    # ---- phase A: node tables (independent of edge packing) -> async upload.
    # Dispatch each device_put as soon as its quantized table is ready so the
    # tunnel transfer overlaps the remaining host-side quantization + packing.
    gscale = float(np.abs(v).max()) / 127.0
    kscale = (np.abs(k).max(axis=1, keepdims=True) / 127.0 + 1e-30).astype(np.float32)
    kv8 = np.empty((N, 2 * HID), np.int8)
    kv8[:, :HID] = np.rint(k / kscale)
    kv8[:, HID:] = np.rint(v * (1.0 / gscale))
    kv8_dev = jax.device_put(kv8, zspec)
    ksc_dev = jax.device_put(kscale.astype(np.float16), zspec)

    qscale = (np.abs(q).max(axis=1, keepdims=True) / 127.0 + 1e-30).astype(np.float32)
    qq = np.rint(q / qscale).astype(np.int8)
    qs_all = np.zeros((NCORES * MAXQ, HID), np.int8)
    qsc_all = np.zeros((NCORES * MAXQ, 1), np.float16)
    qs_all.reshape(NCORES, MAXQ, HID)[:, :SH] = qq.reshape(NCORES, SH, HID)
    qsc_all.reshape(NCORES, MAXQ, 1)[:, :SH] = \
        qscale.astype(np.float16).reshape(NCORES, SH, 1)
    qs_dev = jax.device_put(qs_all, zspec)
    qsc_dev = jax.device_put(qsc_all, zspec)

    # ---- phase B: edge packing (overlaps with phase-A upload)
    per_core, Gmax = _pack(e[0], e[1])
    if Gmax != Gmax_built:   # unexpected input distribution: rebuild
        nc = _build(Gmax)
        _cache["runner"] = (_make_runner(nc), Gmax)
        (run, zspec), Gmax_built = _cache["runner"]

    concat = {
        "kv8": kv8_dev, "ksc": ksc_dev, "qs": qs_dev, "qsc": qsc_dev,
        "srcs": np.concatenate([pc["srcs"] for pc in per_core], axis=0),
        "dstl": np.concatenate([pc["dstl"] for pc in per_core], axis=0),
        "outr": np.concatenate([pc["outr"] for pc in per_core], axis=0),
        "gbase": np.concatenate([pc["gbase"] for pc in per_core], axis=0),
    }
    outs = run(concat)
    xq = np.asarray(outs["xout"]).reshape(NCORES, MAXN + P, HID)
    xs = np.asarray(outs["xsc"]).reshape(NCORES, MAXN + P, 1)

    out = (xq[:, :SH].astype(np.float32)
           * (xs[:, :SH].astype(np.float32) * gscale))
    return out.reshape(1, N, HID)
